# revision 26
# baseline (speedup 1.0000x reference)
"""Expert-parallel MoE BaseLayer kernel for 8 Trainium2 NeuronCores.

Strategy (expert-parallel per the sharding hint; core e holds expert e):
  - Host: route tokens by argmax affinity (float64 numpy), compute the
    sigmoid gate alpha, LayerNorm (+ ln_g/ln_b fold-in), sort tokens by
    expert, pad each expert group to a common capacity C (multiple of 32),
    quantize xln and w1 to TRN fp8_e4m3 (power-of-2 scales, so device
    dequant is exact), and pre-transpose activations to D-major.
  - Device (one Bass program, SPMD over 8 cores):
      ff1: hT = relu(w1^T @ xlnT + b1) via fp8 DoubleRow matmuls
           (2x PE throughput), stationary w1 tiles reused across all
           token chunks so LDWEIGHTS stays hidden; relu+dequant on the
           scalar engine emits bf16 hT.
      ff2: ffnT = w2^T-stationary bf16 matmuls over hT (output D-major,
           so the ragged token tail never wastes a full PE pass).
  - Host: out = x + alpha * (ffn + b2), scattered to original order.
"""

import os

import numpy as np
import ml_dtypes

B, S, D, F, E = 8, 1024, 1024, 4096, 8
T = B * S
EPS = 1e-5
P = 128

SX = 16.0     # xln fp8 scale (power of 2: exact dequant)
SW1 = 1024.0  # w1 fp8 scale
KP8 = 4       # ff1 k-pairs (of 4) done in fp8 DoubleRow; rest bf16

_NC_CACHE = {}
LAST_EXEC_TIME_NS = None
LAST_RESULTS = None


def _balanced_chunks(C, maxc):
    n = -(-C // maxc)
    base = (C // n) // 16 * 16
    sizes = [base] * (n - 1) + [C - base * (n - 1)]
    assert sum(sizes) == C and all(0 < s <= maxc for s in sizes)
    return sizes


def _build_nc(C, kp8):
    import concourse.tile as tile
    from concourse import bacc, mybir
    from concourse.bass import ts

    f32 = mybir.dt.float32
    bf16 = mybir.dt.bfloat16
    f8 = mybir.dt.float8e4
    DR = mybir.MatmulPerfMode.DoubleRow

    KD = D // P          # 8 k-tiles over D
    MF = F // P          # 32 f-tiles over F
    k8 = 2 * kp8         # k-tiles carried in fp8
    kbf = KD - k8        # k-tiles carried in bf16
    chunks1 = _balanced_chunks(C, 256)   # ff1 moving chunks (DoubleRow <=256)
    chunks2 = _balanced_chunks(C, 512)   # ff2 moving chunks (bf16 <=512)
    dq = 1.0 / (SX * SW1)

    NQ = 8  # w1 arrives in F-octiles (host pre-packed q-major: 4KB packets)
    FQ = F // NQ
    nc = bacc.Bacc()
    # All inputs host-packed partition-major: every DMA moves large
    # per-partition-contiguous packets (cold DMA engines are
    # descriptor-latency-bound, so fewer/bigger packets rule the ramp).
    # The ramp-critical data (x8 + w1 octile 0) is co-packed per k-pair so
    # each piece is ONE 128-packet DMA: first matmul unblocks ~2x earlier.
    RW = C + FQ  # combined per-k row: x8 k-row (C) | w18 octile-0 k-row (FQ)
    if k8:
        r8_in = nc.declare_dram_parameter("r8p", [P, k8 * RW], f8, isOutput=False)
        w18_in = nc.declare_dram_parameter(
            "w18p", [P, (NQ - 1) * k8 * FQ], f8, isOutput=False
        )
    if kbf:
        xb_in = nc.declare_dram_parameter("xbp", [P, kbf * C], bf16, isOutput=False)
        w1b_in = nc.declare_dram_parameter("w1bp", [P, NQ * kbf * FQ], bf16, isOutput=False)
    w2_in = nc.declare_dram_parameter("w2p", [P, MF * D], bf16, isOutput=False)
    b1_in = nc.declare_dram_parameter("b1t", [P, MF], f32, isOutput=False)
    out_ext = nc.declare_dram_parameter("outT", [D, C], bf16, isOutput=True)

    if k8:
        r8_v = r8_in[:].rearrange("p (k c) -> p k c", k=k8)
        w18_v = w18_in[:].rearrange("p (q k c) -> p q k c", q=NQ - 1, k=k8)
    if kbf:
        xb_v = xb_in[:].rearrange("p (k c) -> p k c", k=kbf)
        w1b_v = w1b_in[:].rearrange("p (q k c) -> p q k c", q=NQ, k=kbf)
    w2_v = w2_in[:].rearrange("p (k d) -> p k d", k=MF)
    out_v = out_ext[:].rearrange("(k p) c -> k p c", p=P)

    with tile.TileContext(nc) as tc:
        from contextlib import ExitStack

        with ExitStack() as ctx:
            singles = ctx.enter_context(tc.tile_pool(name="singles", bufs=1))
            ps_pool = ctx.enter_context(tc.tile_pool(name="ps", bufs=8, space="PSUM"))

            b1_sb = singles.tile([P, MF], f32)

            # PE p-state warmup: the first ~3us of PE work runs at half
            # clock, and the PE is idle waiting for DMA until ~10us anyway.
            # Fill that window with dependency-free dummy matmuls so the
            # first real matmuls run at full speed.
            warm_sb = singles.tile([P, 2, P], f8)
            nc.vector.memset(warm_sb, 0.0)
            for i in range(64):
                wps = ps_pool.tile([P, 512], f32, tag="ps", name=f"warm_{i}")
                nc.tensor.matmul(
                    wps[:, :P],
                    lhsT=warm_sb[:],
                    rhs=warm_sb[:],
                    start=True,
                    stop=True,
                    perf_mode=DR,
                )

            # One queue, strict priority order: each dma_start already fans
            # across all 16 DMA engines, so a second queue only contends.
            # Interleaved (x kpair, w1 octile-0 kpair) pieces unblock matmul
            # (m=0, kp=0) after ~400KB; cold DMA engines are descriptor-
            # latency bound, so every piece is per-partition contiguous.
            if k8:
                # r8_sb row k = [x8 k-row (C) | w1-octile0 k-row (FQ)]
                r8_sb = singles.tile([P, k8, RW], f8)
                w18_sb = singles.tile([P, NQ - 1, k8, FQ], f8)
            if kbf:
                xb_sb = singles.tile([P, kbf, C], bf16)
                w1b_sb = singles.tile([P, NQ, kbf, FQ], bf16)
            if k8:
                for kp in range(kp8):
                    nc.sync.dma_start(
                        out=r8_sb[:, 2 * kp:2 * kp + 2, :],
                        in_=r8_v[:, 2 * kp:2 * kp + 2, :],
                    )
            if kbf:
                for k in range(kbf):
                    nc.sync.dma_start(
                        out=xb_sb[:, k, :], in_=xb_v[:, k, :]
                    )
                    nc.sync.dma_start(
                        out=w1b_sb[:, 0, k, :], in_=w1b_v[:, 0, k, :]
                    )
            nc.sync.dma_start(out=b1_sb[:], in_=b1_in[:])
            for q0, q1 in ((0, 1), (1, 3), (3, 7)):
                if k8:
                    nc.sync.dma_start(
                        out=w18_sb[:, q0:q1], in_=w18_v[:, q0:q1]
                    )
            if kbf:
                for q0, q1 in ((1, 2), (2, 4), (4, 8)):
                    nc.sync.dma_start(
                        out=w1b_sb[:, q0:q1], in_=w1b_v[:, q0:q1]
                    )
            w2_sb = singles.tile([P, MF, D], bf16)
            for h in range(2):
                HK = MF // 2
                nc.sync.dma_start(
                    out=w2_sb[:, h * HK:(h + 1) * HK, :],
                    in_=w2_v[:, h * HK:(h + 1) * HK, :],
                )

            hT_sb = singles.tile([P, MF, C], bf16)
            oT_sb = singles.tile([P, KD, C], bf16)

            # --- ff1: hT[f, t] = relu(dq * (w1q^T @ xlnq) + b1) ---------
            n_mm1 = kp8 + kbf  # matmuls per psum group
            for m in range(MF):
                banks = [
                    ps_pool.tile([P, 512], f32, tag="ps", name=f"ps1_{m}_{i}")
                    for i in range(len(chunks1))
                ]
                mm = 0
                mq, mj = divmod(m, FQ // P)
                for kp in range(kp8):
                    if mq == 0:
                        w_sl = r8_sb[
                            :, 2 * kp:2 * kp + 2, C + mj * P:C + (mj + 1) * P
                        ]
                    else:
                        w_sl = w18_sb[:, mq - 1, 2 * kp:2 * kp + 2, ts(mj, P)]
                    c0 = 0
                    for ci, Cc in enumerate(chunks1):
                        nc.tensor.matmul(
                            banks[ci][:, :Cc],
                            lhsT=w_sl,
                            rhs=r8_sb[:, 2 * kp:2 * kp + 2, c0:c0 + Cc],
                            start=(mm == 0),
                            stop=(mm == n_mm1 - 1),
                            perf_mode=DR,
                        )
                        c0 += Cc
                    mm += 1
                for k in range(kbf):
                    c0 = 0
                    for ci, Cc in enumerate(chunks1):
                        nc.tensor.matmul(
                            banks[ci][:, :Cc],
                            lhsT=w1b_sb[:, mq, k, ts(mj, P)],
                            rhs=xb_sb[:, k, c0:c0 + Cc],
                            start=(mm == 0),
                            stop=(mm == n_mm1 - 1),
                        )
                        c0 += Cc
                    mm += 1
                c0 = 0
                for ci, Cc in enumerate(chunks1):
                    nc.scalar.activation(
                        out=hT_sb[:, m, c0:c0 + Cc],
                        in_=banks[ci][:, :Cc],
                        func=mybir.ActivationFunctionType.Relu,
                        bias=b1_sb[:, m:m + 1],
                        scale=dq,
                    )
                    c0 += Cc

            # --- ff2: ffnT[d, t] = w2^T @ hT (bf16, w2 stationary) ------
            # The final d-tile uses finer chunks so the tail evac+DMA
            # drain after the last matmul is short.
            for d in range(KD):
                chs = chunks2 if d < KD - 1 else _balanced_chunks(C, 224)
                banks = [
                    ps_pool.tile([P, 512], f32, tag="ps", name=f"ps2_{d}_{i}")
                    for i in range(len(chs))
                ]
                for k in range(MF):
                    c0 = 0
                    for ci, Cc in enumerate(chs):
                        nc.tensor.matmul(
                            banks[ci][:, :Cc],
                            lhsT=w2_sb[:, k, ts(d, P)],
                            rhs=hT_sb[:, k, c0:c0 + Cc],
                            start=(k == 0),
                            stop=(k == MF - 1),
                        )
                        c0 += Cc
                c0 = 0
                for ci, Cc in enumerate(chs):
                    nc.vector.tensor_copy(
                        out=oT_sb[:, d, c0:c0 + Cc], in_=banks[ci][:, :Cc]
                    )
                    nc.sync.dma_start(
                        out=out_v[d][:, c0:c0 + Cc], in_=oT_sb[:, d, c0:c0 + Cc]
                    )
                    c0 += Cc

    nc.compile()
    return nc


def _get_nc(C, kp8):
    key = (C, kp8)
    if key not in _NC_CACHE:
        _NC_CACHE[key] = _build_nc(C, kp8)
    return _NC_CACHE[key]


def _q8(a, scale):
    return np.clip(
        np.asarray(a, np.float32) * scale, -240.0, 240.0
    ).astype(ml_dtypes.float8_e4m3)


def kernel(input_features, centroids, ln_g, ln_b, w1, b1, w2, b2):
    global LAST_EXEC_TIME_NS, LAST_RESULTS
    from concourse.bass_utils import run_bass_kernel_spmd

    x = np.asarray(input_features, dtype=np.float32)
    cen = np.asarray(centroids, dtype=np.float32)
    ln_g = np.asarray(ln_g, dtype=np.float32)
    ln_b = np.asarray(ln_b, dtype=np.float32)
    w1 = np.asarray(w1, dtype=np.float32)
    b1 = np.asarray(b1, dtype=np.float32)
    w2 = np.asarray(w2, dtype=np.float32)
    b2 = np.asarray(b2, dtype=np.float32)

    xf = x.reshape(-1, D)
    n_tok = xf.shape[0]

    # host routing (float64: top-2 gaps are far above fp32 matmul noise)
    aff = xf.astype(np.float64) @ cen.T.astype(np.float64)
    eid = np.argmax(aff, axis=-1)
    dots = np.einsum(
        "td,td->t", xf.astype(np.float64), cen[eid].astype(np.float64)
    )
    alpha = (1.0 / (1.0 + np.exp(-dots))).astype(np.float32)

    # host LayerNorm + per-token gamma/beta (exact, fp32)
    mu = xf.mean(axis=-1, keepdims=True, dtype=np.float64)
    var = np.square(xf - mu).mean(axis=-1, keepdims=True, dtype=np.float64)
    xln = ((xf - mu) / np.sqrt(var + EPS)).astype(np.float32)
    xln = xln * ln_g[eid] + ln_b[eid]

    idx = [np.nonzero(eid == e)[0] for e in range(E)]
    max_cnt = max(1, max(len(i) for i in idx))
    C = ((max_cnt + 31) // 32) * 32

    k8 = 2 * KP8
    nc = _get_nc(C, KP8)

    NQ = 8
    FQ = F // NQ

    def _pack_w1(w, nk):
        # [nk*P, F] -> [P, NQ*nk*FQ] with w_p[p, q, k, c] = w[k*P+p, q*FQ+c]
        return np.ascontiguousarray(
            w.reshape(nk, P, NQ, FQ).transpose(1, 2, 0, 3).reshape(P, -1)
        )

    def _pack_k(a, nk):
        # [nk*P, N] -> [P, nk*N] with a_p[p, k, c] = a[k*P+p, c]
        return np.ascontiguousarray(
            a.reshape(nk, P, -1).transpose(1, 0, 2).reshape(P, -1)
        )

    in_maps = []
    for e in range(E):
        xs = np.zeros((C, D), dtype=np.float32)
        xs[: len(idx[e])] = xln[idx[e]]
        xsT = np.ascontiguousarray(xs.T)  # [D, C]
        im = {
            "w2p": _pack_k(w2[e].astype(ml_dtypes.bfloat16), F // P),
            "b1t": np.ascontiguousarray(b1[e].reshape(F // P, P).T),
        }
        if k8:
            xq = _q8(xsT[: k8 * P], SX)
            w1q = _q8(w1[e][: k8 * P], SW1)
            im["r8p"] = np.ascontiguousarray(
                np.concatenate(
                    [
                        xq.reshape(k8, P, C).transpose(1, 0, 2),
                        w1q[:, :FQ].reshape(k8, P, FQ).transpose(1, 0, 2),
                    ],
                    axis=2,
                ).reshape(P, -1)
            )
            im["w18p"] = np.ascontiguousarray(
                w1q[:, FQ:]
                .reshape(k8, P, NQ - 1, FQ)
                .transpose(1, 2, 0, 3)
                .reshape(P, -1)
            )
        if k8 < 8:
            im["xbp"] = _pack_k(
                (xsT[k8 * P:] * SX).astype(ml_dtypes.bfloat16), 8 - k8
            )
            im["w1bp"] = _pack_w1(
                (w1[e][k8 * P:] * SW1).astype(ml_dtypes.bfloat16), 8 - k8
            )
        in_maps.append(im)

    want_trace = bool(int(os.environ.get("KERNEL_TRACE", "0")))
    if not want_trace:
        os.environ["BASS_NEVER_TRACE"] = "1"
    res = run_bass_kernel_spmd(
        nc,
        in_maps,
        list(range(E)),
        trace=want_trace,
    )
    LAST_EXEC_TIME_NS = res.exec_time_ns
    LAST_RESULTS = res

    out_full = np.empty((n_tok, D), dtype=np.float32)
    for e in range(E):
        cnt = len(idx[e])
        if cnt:
            ffn = res.results[e]["outT"].astype(np.float32).T[:cnt]  # [cnt, D]
            out_full[idx[e]] = (
                xf[idx[e]] + alpha[idx[e], None] * (ffn + b2[e])
            )
    return out_full.reshape(x.shape)
